# revision 44
# baseline (speedup 1.0000x reference)
"""Sliding-window GQA causal self-attention (ALiBi) Trainium2 Bass kernel.

Problem: B=2, T=4096, C=1024, H=16, HKV=4 (GQA G=4), D=64, window W=512,
fused qkv projection + sliding-window attention + output projection.

Sharding: data-parallel over (batch x T/4) -> 8 cores. Each core computes
1024 query rows of one batch plus a 512-row k/v halo. No collectives.

Per-core dataflow (projection/PV/out-proj matmuls in bf16, score matmuls
in float32r so the integer-valued ALiBi augmentation stays exact):
  - x and the weights arrive host-interleaved to the exact SBUF layout so
    loads are few wide 2D DMAs
  - qT computed transposed into a packed layout [kv][qb*512 + g*128 + q]
    so one N=512 score matmul covers all 4 GQA heads of a kv group;
    kT computed transposed per kv head; v natural (bf16) with a ones
    column appended per (chunk, kv) block
  - scores computed TRANSPOSED: sT[keys, q] = kT_aug^T @ qT_aug per
    128-key chunk (K=67; 3 augmentation rows fold the ALiBi bias and
    left-edge -1e9 penalty) - no PE transposes
  - softmax without max-subtraction (scores are N(0,~6.5); exp cannot
    overflow at <13 sigma)
  - the whole attention runs as a software-pipelined slot stream:
    slot k = [scores(unit k) j0..j3 + exps] [PV(unit k-1) j0..j3]
    [scale(unit k-2)] [scores k j4] [PV k-1 j4] + filler (projection /
    out-proj groups, ~2 per slot), so the PE never blocks on the
    activation engine's exp and projection work streams through the
    attention phase instead of alternating with it
  - score chunks rotate through a 4-tile PSUM pool (independent subtile
    deps keep score/exp/PV of neighboring units fully decoupled);
    window masks are 0/1 bf16 multiplies on the Pool engine post-exp
    (j0) and a bf16 -1e9 add pre-exp (j4, DVE)
  - PV accumulates oT[65, 512] over 5 key chunks; the v ones column
    makes row 64 the softmax denominator for free
  - normalization AFTER PV: one [1,512] reciprocal, replicated across 64
    partitions by a stride-0 SBUF DMA issued from the SP queue one slot
    ahead of its use, one [64,512] multiply into bf16 attnT per
    g-parity; deferred two slots so nothing stalls
  - out = attnT.T @ wo (bf16) per q-block, split in two N=512 stages
    that flow through the same filler stream; the last block ships its
    odd-g slab incrementally and broadcasts its final 1/s via a PE
    outer product so only a short serial chain trails the last PV
  - all input loads issue from the SP queue (act-queue DMA dispatches
    would stall the projection copies ~7us at startup); the first x
    slice and wk arrive in interleaved halves so the first k-projection
    starts ~1.5us earlier
"""

import math
from collections import deque
from contextlib import ExitStack

import numpy as np
import ml_dtypes

import concourse.bass as bass
from concourse import bacc
import concourse.mybir as mybir
import concourse.tile as tile
from concourse.bass_utils import run_bass_kernel_spmd

F32 = mybir.dt.float32
F32R = mybir.dt.float32r
BF16 = mybir.dt.bfloat16

B, T, C = 2, 4096, 1024
H, HKV, G, D = 16, 4, 4, 64
W = 512
NCORES = 8
RT = 1024              # own query rows per core
KR = RT + W            # k/v slab rows (512 halo + 1024 own)
NQB = RT // 128        # 8 q-blocks of 128
NKC = KR // 128        # 12 k-chunks of 128
NTS = KR // 256        # 6 projection time slices
SCALE = D ** -0.5      # 0.125, exact power of two
NEG = -1e9
VW = 65                # v block width: 64 features + ones column




def alibi_slopes(n_head: int) -> np.ndarray:
    def slopes_power_of_2(n):
        start = 2.0 ** (-(2.0 ** (-(math.log2(n) - 3))))
        return [start * start ** i for i in range(n)]

    if float(math.log2(n_head)).is_integer():
        s = slopes_power_of_2(n_head)
    else:
        closest = 2 ** math.floor(math.log2(n_head))
        s = slopes_power_of_2(closest)
        s2 = slopes_power_of_2(2 * closest)
        s += s2[0::2][: n_head - closest]
    return np.array(s, dtype=np.float32)


def build_nc(loop: int = 1) -> bacc.Bacc:
    nc = bacc.Bacc("TRN2", target_bir_lowering=False)

    # host-interleaved: xs[p, ts*2048 + cc*256 + j] = x[ts*256+j, cc*128+p]
    xs = nc.dram_tensor("xs", [128, NTS * 2048], BF16, kind="ExternalInput")
    wqh = nc.dram_tensor("wqh", [128, 8 * 1024], BF16, kind="ExternalInput")
    wkh = nc.dram_tensor("wkh", [128, 8 * 256], BF16, kind="ExternalInput")
    wvh = nc.dram_tensor("wvh", [128, 8 * 256], BF16, kind="ExternalInput")
    woh = nc.dram_tensor("woh", [128, 8 * 1024], BF16, kind="ExternalInput")
    qaug = nc.dram_tensor("qaug", [3, HKV * G * RT], F32R, kind="ExternalInput")
    kaug = nc.dram_tensor("kaug", [3, HKV * KR], F32R, kind="ExternalInput")
    m0q4 = nc.dram_tensor("m0q4", [128, 512], BF16, kind="ExternalInput")
    w4q4 = nc.dram_tensor("w4q4", [128, 512], BF16, kind="ExternalInput")
    out = nc.dram_tensor("out", [RT, C], F32, kind="ExternalOutput")

    Exp = mybir.ActivationFunctionType.Exp

    with tile.TileContext(nc) as tc:
      for _rep in range(loop):
        with ExitStack() as ctx:
            persist = ctx.enter_context(tc.tile_pool(name="persist", bufs=1))

            # packed qT, one slab: rows 0:64 data, 64:67 aug;
            # col = kv*4096 + qb*512 + g*128 + q
            qT = persist.tile([128, HKV * NQB * 512], F32R)
            kT = persist.tile([128, HKV * KR], F32R)
            # v natural bf16, kc-major: [t-in-chunk, kc*260 + kv*65 + (d|ones)]
            vsl = persist.tile([128, NKC * HKV * VW], BF16)
            wo_sb = persist.tile([128, 8 * 1024], BF16)
            wq_all = persist.tile([128, 8 * 1024], BF16)
            wk_sb = persist.tile([128, 8 * 256], BF16)
            wv_sb = persist.tile([128, 8 * 256], BF16)
            m0_sb = persist.tile([128, 512], BF16)
            w4_sb = persist.tile([128, 512], BF16)
            # ones row on partition 64 (PE matmul needs lhsT/rhs aligned)
            ones64 = persist.tile([65, 64], F32)
            nc.vector.memset(ones64[64:65, :], 1.0)

            vones = bass.AP(tensor=vsl.tensor, offset=vsl.offset + 64,
                            ap=[list(vsl.ap[0]), [HKV * VW, NKC], [VW, HKV]])
            nc.vector.memset(vones, 1.0)

            xTp = ctx.enter_context(tc.tile_pool(name="xTp", bufs=3))
            stp = ctx.enter_context(tc.tile_pool(name="stp", bufs=4))
            pTp = ctx.enter_context(tc.tile_pool(name="pTp", bufs=12))
            atp = ctx.enter_context(tc.tile_pool(name="atp", bufs=3))
            obp = ctx.enter_context(tc.tile_pool(name="obp", bufs=3))
            sgp = ctx.enter_context(tc.tile_pool(name="sgp", bufs=3))
            rsp = ctx.enter_context(tc.tile_pool(name="rsp", bufs=4))
            bcp = ctx.enter_context(tc.tile_pool(name="bcp", bufs=3))
            psA = ctx.enter_context(tc.tile_pool(name="psA", bufs=2, space="PSUM"))
            psS = ctx.enter_context(tc.tile_pool(name="psS", bufs=4, space="PSUM"))
            psO = ctx.enter_context(tc.tile_pool(name="psO", bufs=2, space="PSUM"))

            def load_slice(ts):
                xTt = xTp.tile([128, 8 * 256], BF16, tag="xts")
                nc.sync.dma_start(xTt, xs[:, ts * 2048:(ts + 1) * 2048])
                return xTt

            # ---------------- projection groups ----------------
            def proj_groups(ts, xTt):
                t0 = ts * 256
                groups = []

                def k_group(fi):
                    def emit():
                        pst = psA.tile([128, 512], F32, tag="ps")
                        ps = pst[:, 0:256]
                        for cc in range(8):
                            nc.tensor.matmul(
                                ps,
                                lhsT=wk_sb[:, cc * 256 + fi * 128:
                                           cc * 256 + (fi + 1) * 128],
                                rhs=xTt[:, cc * 256:(cc + 1) * 256],
                                start=(cc == 0), stop=(cc == 7))
                        kv0, kv1 = 2 * fi, 2 * fi + 1
                        nc.scalar.copy(kT[0:64, kv0 * KR + t0:kv0 * KR + t0 + 256],
                                       ps[0:64, :])
                        st = stp.tile([128, 256], F32R, tag="st")
                        nc.vector.tensor_copy(st[64:128, :], ps[64:128, :])
                        nc.sync.dma_start(kT[0:64, kv1 * KR + t0:kv1 * KR + t0 + 256],
                                          st[64:128, :])
                    return emit

                def v_group(tki):
                    def emit():
                        kc = ts * 2 + tki
                        psvt = psA.tile([128, 512], F32, tag="ps")
                        psv = psvt[:, 0:256]
                        for cc in range(8):
                            nc.tensor.matmul(
                                psv,
                                lhsT=xTt[:, cc * 256 + tki * 128:
                                         cc * 256 + (tki + 1) * 128],
                                rhs=wv_sb[:, cc * 256:(cc + 1) * 256],
                                start=(cc == 0), stop=(cc == 7))
                        vdst = bass.AP(tensor=vsl.tensor,
                                       offset=vsl.offset + kc * HKV * VW,
                                       ap=[list(vsl.ap[0]), [VW, HKV], [1, 64]])
                        nc.vector.tensor_copy(
                            vdst, psv.rearrange("p (a b) -> p a b", b=64))
                    return emit

                def q_group(kv, fi):
                    def emit():
                        toff = t0 - 512
                        qb0 = toff // 128
                        qbase = kv * 4096 + qb0 * 512
                        pst = psA.tile([128, 512], F32, tag="ps")
                        ps = pst[:, 0:256]
                        for cc in range(8):
                            nc.tensor.matmul(
                                ps,
                                lhsT=wq_all[:, cc * 1024 + kv * 256 + fi * 128:
                                            cc * 1024 + kv * 256 + (fi + 1) * 128],
                                rhs=xTt[:, cc * 256:(cc + 1) * 256],
                                start=(cc == 0), stop=(cc == 7))
                        ge, go = 2 * fi, 2 * fi + 1
                        dste = bass.AP(
                            tensor=qT.tensor,
                            offset=qT.offset + qbase + ge * 128,
                            ap=[[qT.ap[0][0], 64], [512, 2], [1, 128]])
                        nc.scalar.copy(
                            dste, ps[0:64, :].rearrange("p (a b) -> p a b", b=128))
                        st = stp.tile([128, 256], F32R, tag="st")
                        nc.vector.tensor_copy(st[64:128, :], ps[64:128, :])
                        dsto = bass.AP(
                            tensor=qT.tensor,
                            offset=qT.offset + qbase + go * 128,
                            ap=[[qT.ap[0][0], 64], [512, 2], [1, 128]])
                        nc.sync.dma_start(dsto, st[64:128, :].rearrange(
                            "p (a b) -> p a b", b=128))
                    return emit

                for fi in range(2):
                    groups.append(k_group(fi))
                for tki in range(2):
                    groups.append(v_group(tki))
                if ts >= 2:
                    for kv in range(HKV):
                        for fi in range(2):
                            groups.append(q_group(kv, fi))
                return groups

            def pair(src, off):
                s64 = src[0:64, :]
                return bass.AP(tensor=s64.tensor, offset=s64.offset + off,
                               ap=[list(s64.ap[0]), [256, 2], [1, 128]])

            # ---------------- attention slot pipeline ----------------
            blocks = {}
            filler_q = deque()

            def scores_a(st):
                # chunks j0..j3, exp right behind each score matmul
                qb, kv = st["qb"], st["kv"]
                pcs = st["pcs"]
                for j in range(4):
                    ck = qb + j
                    ps = psS.tile([128, 512], F32, tag="sc")
                    nc.tensor.matmul(
                        ps,
                        lhsT=kT[0:67, kv * KR + ck * 128:kv * KR + (ck + 1) * 128],
                        rhs=qT[0:67, kv * 4096 + qb * 512:kv * 4096 + (qb + 1) * 512],
                        start=True, stop=True)
                    pc = pTp.tile([128, 512], BF16, tag="pc")
                    nc.scalar.activation(pc, ps, Exp, bias=0.0)
                    if j == 0:
                        # chunk j0 window mask (0/1), on the Pool engine
                        nc.gpsimd.tensor_mul(pc, pc, m0_sb)
                    pcs.append(pc)

            def scores_b(st):
                # diagonal chunk j4: mask must precede exp (positive ALiBi
                # bias on invalid cells would overflow exp)
                qb, kv = st["qb"], st["kv"]
                ps = psS.tile([128, 512], F32, tag="sc")
                nc.tensor.matmul(
                    ps,
                    lhsT=kT[0:67, kv * KR + (qb + 4) * 128:kv * KR + (qb + 5) * 128],
                    rhs=qT[0:67, kv * 4096 + qb * 512:kv * 4096 + (qb + 1) * 512],
                    start=True, stop=True)
                nc.vector.tensor_add(ps, ps, w4_sb)
                pc = pTp.tile([128, 512], BF16, tag="pc")
                nc.scalar.activation(pc, ps, Exp, bias=0.0)
                st["pcs"].append(pc)

            def pv_a(st):
                qb, kv = st["qb"], st["kv"]
                po = psO.tile([65, 512], F32, tag="ot")
                for j in range(4):
                    base = (qb + j) * HKV * VW + kv * VW
                    nc.tensor.matmul(po, lhsT=vsl[:, base:base + VW],
                                     rhs=st["pcs"][j], start=(j == 0), stop=False)
                st["po"] = po

            def pv_b(st):
                qb, kv = st["qb"], st["kv"]
                po = st["po"]
                base = (qb + 4) * HKV * VW + kv * VW
                nc.tensor.matmul(po, lhsT=vsl[:, base:base + VW],
                                 rhs=st["pcs"][4], start=False, stop=True)
                rs = rsp.tile([65, 512], F32, tag="rs")
                with nc.allow_low_precision(reason="fp32 out"):
                    nc.vector.reciprocal(rs[64:65, :], po[64:65, :])
                if (qb, kv) == (7, 3):
                    # final unit: broadcast 1/s via a PE outer product plus
                    # an act copy to SBUF (the scale mul may read only one
                    # PSUM operand) instead of the ~2.4us DMA round-trip,
                    # shortening the tail's serial chain
                    bct = psS.tile([128, 512], F32, tag="sc")
                    nc.tensor.matmul(bct[0:64, :], lhsT=ones64[64:65, :],
                                     rhs=rs[64:65, :], start=True, stop=True)
                    bcs = bcp.tile([64, 512], F32, tag="bcs")
                    nc.scalar.copy(bcs, bct[0:64, :])
                else:
                    # broadcast 1/s across 64 partitions one slot early
                    # (SP queue)
                    bcs = bcp.tile([64, 512], F32, tag="bcs")
                    r64 = rs[64:65, :]
                    brd = bass.AP(tensor=r64.tensor, offset=r64.offset,
                                  ap=[list(r64.ap[0]), [0, 64], [1, 512]])
                    nc.sync.dma_start(bcs, brd)
                st["bcs"] = bcs

            def op_stage(qb, ec):
                def emit():
                    bst = blocks[qb]
                    at, ob = bst["at"], bst["ob"]
                    pf = psA.tile([128, 512], F32, tag="ps")
                    for cc in range(8):
                        nc.tensor.matmul(
                            pf,
                            lhsT=at[:, cc * 128:(cc + 1) * 128],
                            rhs=wo_sb[:, cc * 1024 + ec * 512:
                                      cc * 1024 + ec * 512 + 512],
                            start=(cc == 0), stop=(cc == 7))
                    nc.vector.tensor_copy(ob[:, ec * 512:(ec + 1) * 512], pf)
                    nc.sync.dma_start(
                        out[qb * 128:(qb + 1) * 128, ec * 512:(ec + 1) * 512],
                        ob[:, ec * 512:(ec + 1) * 512])
                return emit



            def emit_scale(st):
                qb, kv = st["qb"], st["kv"]
                if kv == 0:
                    blocks[qb] = dict(
                        at=atp.tile([128, 8 * 128], BF16, tag="at",
                                    name=f"at{qb}"),
                        sg4=sgp.tile([64, 1024], BF16, tag="sg",
                                     name=f"sg{qb}"),
                        ob=obp.tile([128, 1024], F32, tag="ob",
                                    name=f"ob{qb}"))
                bst = blocks[qb]
                at, sg4 = bst["at"], bst["sg4"]
                po, bcs = st["po"], st["bcs"]
                # even g -> attnT rows 0:64 directly; odd g into the sg4 slab
                nc.vector.tensor_mul(at[0:64, kv * 256:kv * 256 + 256],
                                     pair(po, 0), pair(bcs, 0))
                nc.vector.tensor_mul(sg4[:, kv * 256:kv * 256 + 256],
                                     pair(po, 128), pair(bcs, 128))
                if qb == 7:
                    # last block: ship each kv's odd-g slab slice as it is
                    # produced so only a tiny [64,256] DMA trails the final
                    # scale on the drain's critical chain
                    nc.sync.dma_start(at[64:128, kv * 256:kv * 256 + 256],
                                      sg4[:, kv * 256:kv * 256 + 256])
                    if kv == 3:
                        pipe["defer"].append((2, op_stage(qb, 0)))
                        pipe["defer"].append((2, op_stage(qb, 1)))
                elif kv == 3:
                    nc.sync.dma_start(at[64:128, :], sg4)
                    # defer one slot so the out-proj matmuls never wait
                    # on the in-flight sg4 partition-shift DMA
                    pipe["defer"].append((2, op_stage(qb, 0)))
                    pipe["defer"].append((2, op_stage(qb, 1)))

            pipe = {"prev": None, "prev2": None, "defer": []}

            def slot(u, budget=2):
                st = dict(qb=u[0], kv=u[1], pcs=[]) if u is not None else None
                prev, prev2 = pipe["prev"], pipe["prev2"]
                while pipe["defer"]:
                    filler_q.append(pipe["defer"].pop(0))
                if st:
                    scores_a(st)
                if prev:
                    pv_a(prev)
                if prev2:
                    emit_scale(prev2)
                if st:
                    scores_b(st)
                if prev:
                    pv_b(prev)
                while filler_q and budget > 0:
                    cost, fn = filler_q.popleft()
                    budget -= cost
                    fn()
                pipe["prev2"], pipe["prev"] = prev, st

            # ---------------- schedule ----------------
            # all loads go through the SP queue: DMA dispatches on the act
            # queue would delay the projection copies behind ~7us of
            # descriptor setup
            # first x slice and wk arrive in interleaved cc-halves so both
            # k_groups' cc 0..3 matmuls can start after only half the data
            xT0 = xTp.tile([128, 8 * 256], BF16, tag="xts")
            nc.sync.dma_start(xT0[:, 0:1024], xs[:, 0:1024])
            nc.sync.dma_start(wk_sb[:, 0:1024], wkh[:, 0:1024])
            nc.sync.dma_start(xT0[:, 1024:2048], xs[:, 1024:2048])
            nc.sync.dma_start(wk_sb[:, 1024:2048], wkh[:, 1024:2048])
            xts = [xT0]
            nc.sync.dma_start(wv_sb, wvh[:, :])
            xts += [load_slice(1), load_slice(2)]
            for h in range(4):
                nc.sync.dma_start(wq_all[:, h * 2048:(h + 1) * 2048],
                                  wqh[:, h * 2048:(h + 1) * 2048])
            nc.sync.dma_start(m0_sb, m0q4[:, :])
            nc.sync.dma_start(w4_sb, w4q4[:, :])
            nc.sync.dma_start(qT[64:67, :], qaug[:, :])
            nc.sync.dma_start(kT[64:67, :], kaug[:, :])
            for g in proj_groups(0, xts[0]):
                g()
            for g in proj_groups(1, xts[1]):
                g()
            for h in range(2):
                nc.sync.dma_start(wo_sb[:, h * 4096:(h + 1) * 4096],
                                  woh[:, h * 4096:(h + 1) * 4096])
            for g in proj_groups(2, xts[2]):
                g()

            for p in range(4):
                if p < 3:
                    xts.append(load_slice(3 + p))
                    for g in proj_groups(3 + p, xts[3 + p]):
                        filler_q.append((1, g))
                for qb in (2 * p, 2 * p + 1):
                    for kv in range(HKV):
                        # hold back the last out-proj stages so the drain
                        # below keeps the PE warm through the final
                        # normalize/ship chain (idle would also drop the
                        # PE out of its max p-state)
                        bgt = 0 if (qb, kv) >= (7, 2) else 2
                        slot((qb, kv), budget=bgt)
            # drain the pipeline: PV + scale of the last two units,
            # interleaved with the reserved out-proj stages
            slot(None, budget=2)
            slot(None, budget=99)
            while pipe["defer"]:
                filler_q.append(pipe["defer"].pop(0))
            while filler_q:
                _, fn = filler_q.popleft()
                fn()

    nc.compile()
    return nc


_NC = None


def _host_inputs(x, wqkv, wo):
    slopes = alibi_slopes(H)  # head h = kv*G + g matches slopes.reshape(HKV, G)

    wqkv_s = np.array(wqkv, dtype=np.float32, copy=True)
    wqkv_s[:, :C] *= SCALE  # exact power-of-two fold of the score scale into wq

    # interleave weights to the SBUF layouts (one wide DMA each)
    wqh = np.ascontiguousarray(
        wqkv_s[:, :C].reshape(8, 128, C).transpose(1, 0, 2)
        .reshape(128, 8 * C)).astype(ml_dtypes.bfloat16)
    wkh = np.ascontiguousarray(
        wqkv_s[:, C:C + 256].reshape(8, 128, 256).transpose(1, 0, 2)
        .reshape(128, 8 * 256)).astype(ml_dtypes.bfloat16)
    wvh = np.ascontiguousarray(
        wqkv_s[:, C + 256:].reshape(8, 128, 256).transpose(1, 0, 2)
        .reshape(128, 8 * 256)).astype(ml_dtypes.bfloat16)
    wob = np.asarray(wo, dtype=np.float32).astype(ml_dtypes.bfloat16)
    woh = np.ascontiguousarray(
        wob.reshape(8, 128, C).transpose(1, 0, 2).reshape(128, 8 * C))

    # packed q augmentation: col = kv*4096 + qb*512 + g*128 + q, t = qb*128 + q
    qaug = np.empty((3, HKV * G * RT), dtype=np.float32)
    cols = np.arange(HKV * G * RT)
    col_kv = cols // 4096
    col_t = ((cols % 4096) // 512) * 128 + (cols % 128)
    col_g = (cols % 512) // 128
    sl = slopes[col_kv * G + col_g]
    qaug[0] = -sl * (col_t + 512.0)
    qaug[1] = sl
    qaug[2] = 1.0

    i = np.arange(KR, dtype=np.float32)
    kaug_base = np.empty((3, KR), dtype=np.float32)
    kaug_base[0] = 1.0
    kaug_base[1] = i
    kaug_base[2] = 0.0

    # transposed-score window masks on the extreme chunks, tiled for 4 g:
    # chunk j=0: valid q < r; chunk j=4: valid q >= r (0/1 multiplies on p)
    r = np.arange(128)[:, None]
    q = np.arange(128)[None, :]
    m0 = np.where(q < r, 1.0, 0.0).astype(ml_dtypes.bfloat16)
    w4 = np.where(q < r, NEG, 0.0).astype(ml_dtypes.bfloat16)
    m0q4 = np.ascontiguousarray(np.tile(m0, (1, 4)))
    w4q4 = np.ascontiguousarray(np.tile(w4, (1, 4)))

    in_maps = []
    for core in range(NCORES):
        b, qq = core // 4, core % 4
        t0 = qq * RT
        xsl = np.zeros((KR, C), dtype=np.float32)
        lo = t0 - W
        if lo < 0:
            xsl[-lo:, :] = x[b, 0:t0 + RT, :]
        else:
            xsl[:, :] = x[b, lo:t0 + RT, :]
        # interleave: xs[p, ts*2048 + cc*256 + j] = xsl[ts*256+j, cc*128+p]
        xs2 = np.ascontiguousarray(
            xsl.reshape(NTS, 256, 8, 128).transpose(3, 0, 2, 1)
            .reshape(128, -1)).astype(ml_dtypes.bfloat16)
        kaug = kaug_base.copy()
        if lo < 0:
            kaug[2, :W] = NEG  # left-edge penalty kills padded keys
        kaug4 = np.ascontiguousarray(np.tile(kaug, (1, HKV)).reshape(3, HKV * KR))
        in_maps.append(dict(xs=xs2, wqh=wqh, wkh=wkh, wvh=wvh, woh=woh,
                            qaug=qaug, kaug=kaug4, m0q4=m0q4, w4q4=w4q4))
    return in_maps


def kernel(x, wqkv, wo):
    global _NC
    if _NC is None:
        _NC = build_nc()
    in_maps = _host_inputs(np.asarray(x), np.asarray(wqkv), np.asarray(wo))
    res = run_bass_kernel_spmd(_NC, in_maps, list(range(NCORES)))
    full = np.empty((B, T, C), dtype=np.float32)
    for core in range(NCORES):
        b, qq = core // 4, core % 4
        full[b, qq * RT:(qq + 1) * RT, :] = res.results[core]["out"]
    return full
